# revision 17
# baseline (speedup 1.0000x reference)
"""ChebConv with spatial attention — Trainium2 Bass kernel.

Problem (reference semantics):
    A   = cheb[None,k] * spatial_attention[b]          # [B,K,N,N]
    rhs = einsum('bkij,btjf->btkif', A, x)             # graph propagation
    out = relu(einsum('btkif,kfo->btio', rhs, Theta))  # per-order linear + sum_k

Shapes: B=16, T=12, N=1024, F_in=F_out=64, K=3 (all fp32).

Strategy:
  * Data-parallel over B: 2 batches per core x 8 NeuronCores. No collectives.
  * Theta applied FIRST:  XW_bk[j,(t,o)] = x[b,t,j,:] @ Theta[k]   (cheap)
    then the 77-GFLOP graph propagation, computed TRANSPOSED so that the
    reusable XW tiles are the stationary operand:
        out'[(t,o), i] = sum_{k,j} XW[j,(t,o)] * AT[j,i]
    with AT[j,i] = chebT[j,i]*attT[j,i] (elementwise, vector engine).
    att/cheb/x are transposed on the host (free layout prep).
  * float32r (tf32) matmuls: 4x faster than fp32 on the PE at free-dim>=256,
    ~3e-4 relative error.  Operands are pre-rounded (host) or rounded on
    write by DVE/ACT, as the hardware requires.
  * XW-phase matmuls are packed in concurrent PE row groups (t, t+6).
  * chebT is streamed from HBM in [128,512] chunks; x, attT, XW resident.
  * out' is stored transposed+permuted; the host un-permutes (numpy).
"""
import numpy as np

import concourse.bass as bass
import concourse.tile as tile
from concourse import bacc, mybir
from concourse.bass_utils import run_bass_kernel_spmd

F32 = mybir.dt.float32
F32R = mybir.dt.float32r

B, T, N, F, K = 16, 12, 1024, 64, 3
NCORES = 8
BPC = B // NCORES          # batches per core
TH = T // 2                # 6 "tl" column chunks of x
TF = T * F                 # 768 = 6 to'-blocks of 128
NJB = N // 128             # 8 j-blocks
THETA_PAD = 512            # block-diag: rows 0:64 -> cols 0:256, rows 64:128 -> cols 256:512


def _round_tf32(a: np.ndarray) -> np.ndarray:
    """Round fp32 -> tf32 (10 mantissa bits), required for float32r operands."""
    u = np.ascontiguousarray(a).view(np.uint32)
    lsb = (u >> np.uint32(13)) & np.uint32(1)
    rounded = (u + np.uint32(0x0FFF) + lsb) & np.uint32(0xFFFFE000)
    return rounded.view(np.float32)


def _build_nc():
    nc = bacc.Bacc("TRN2", target_bir_lowering=False, debug=False, num_devices=NCORES)

    # xt[b, tl, f + 64*th, j] = x[b, th*6+tl, j, f]
    xt_d = nc.dram_tensor("xt", [BPC, TH, 128, N], F32R, kind="ExternalInput")
    att_d = nc.dram_tensor("attT", [BPC, N, N], F32, kind="ExternalInput")
    cheb_d = nc.dram_tensor("chebT", [K, N, N], F32, kind="ExternalInput")
    th_d = nc.dram_tensor("theta", [128, THETA_PAD], F32R, kind="ExternalInput")
    # out'[b, tob, ih, to'-in-block, i-in-half]; to' = tl*128 + th*64 + o
    out_d = nc.dram_tensor("out", [BPC, TH, 2, 128, 512], F32, kind="ExternalOutput")

    with tile.TileContext(nc) as tc:
        with (
            tc.tile_pool(name="const", bufs=1) as const_pool,
            tc.tile_pool(name="xtc", bufs=9) as xtc_pool,
            tc.tile_pool(name="attc", bufs=10) as attc_pool,
            tc.tile_pool(name="xw", bufs=1) as xw_pool,
            tc.tile_pool(name="cheb", bufs=8) as cheb_pool,
            tc.tile_pool(name="atp", bufs=8) as at_pool,
            tc.tile_pool(name="outp", bufs=6) as out_pool,
            tc.tile_pool(name="ps", bufs=4, space="PSUM") as ps_pool,
        ):
            theta_sb = const_pool.tile([128, THETA_PAD], F32R)
            nc.sync.dma_start(theta_sb[:], th_d[:])

            # PE warmup: ~3.5us of dummy matmuls so HAM un-throttles before
            # the real work starts (runs during the xt/att input DMAs).
            warm = ps_pool.tile([128, 1024], F32, tag="ps", name="warm")
            for w in range(10):
                nc.tensor.matmul(
                    warm[:, 0:128],
                    theta_sb[:, 0:128],
                    theta_sb[:, 0:128],
                    start=True,
                    stop=True,
                )

            for lb in range(BPC):
                # chunked input loads (x first: it gates the XW phase)
                xtc = []
                for tl in range(TH):
                    xc = xtc_pool.tile([128, N], F32R, tag="xt", name=f"xt{lb}_{tl}")
                    nc.sync.dma_start(xc[:], xt_d[lb, tl])
                    xtc.append(xc)
                attc = []
                for jb in range(NJB):
                    ac = attc_pool.tile([128, N], F32, tag="att", name=f"att{lb}_{jb}")
                    nc.sync.dma_start(
                        ac[:], att_d[lb, jb * 128 : (jb + 1) * 128, :]
                    )
                    attc.append(ac)

                # ---- XW phase: xw[j, k, jb, to'] with to' = tl*128+th*64+o
                xw_sb = xw_pool.tile([128, K, NJB, TF], F32R, tag="xw", name=f"xw{lb}")
                for tl2 in range(TH // 2):
                    for jb in range(NJB):
                        pxw = ps_pool.tile(
                            [128, 1024], F32, tag="ps", name=f"pxw{lb}_{jb}_{tl2}"
                        )
                        for h in range(2):
                            nc.tensor.matmul(
                                pxw[:, h * 512 : (h + 1) * 512],
                                xtc[tl2 * 2 + h][:, jb * 128 : (jb + 1) * 128],
                                theta_sb[:],
                                start=True,
                                stop=True,
                            )
                        src = pxw[:].rearrange(
                            "p (a t k o) -> p k a t o", a=2, t=2, k=4, o=64
                        )[:, 0:K]
                        dst = xw_sb[:, :, jb, tl2 * 256 : (tl2 + 1) * 256].rearrange(
                            "p k (a t o) -> p k a t o", a=2, t=2
                        )
                        if tl2 == TH // 2 - 1:
                            # final XW pass: half-copies on both engines so the
                            # psum slots drain fast for the main phase
                            nc.scalar.copy(dst[:, :, 0], src[:, :, 0])
                            nc.vector.tensor_copy(dst[:, :, 1], src[:, :, 1])
                        elif jb % 2 == 0:
                            nc.scalar.copy(dst, src)
                        else:
                            nc.vector.tensor_copy(dst, src)
                        last_pxw = pxw

                # keep the PE HAM-warm through the XW->main copy-drain boundary
                for w in range(6):
                    nc.tensor.matmul(
                        last_pxw[:, 0:128],
                        theta_sb[:, 0:128],
                        theta_sb[:, 0:128],
                        start=True,
                        stop=True,
                    )

                # ---- main phase: out'[to', i] = sum_{k,j} XW[j,to']*AT[j,i]
                for ih in range(2):
                    opt2 = [
                        ps_pool.tile([128, 1024], F32, tag="ps", name=f"op_{lb}_{ih}_{i}")
                        for i in range(TH // 2)
                    ]
                    ops = [opt2[i // 2][:, (i % 2) * 512 : (i % 2 + 1) * 512] for i in range(TH)]
                    for k in range(K):
                        for jb in range(NJB):
                            ch = cheb_pool.tile([128, 512], F32, tag="ch")
                            nc.sync.dma_start(
                                ch[:],
                                cheb_d[
                                    k,
                                    jb * 128 : (jb + 1) * 128,
                                    ih * 512 : (ih + 1) * 512,
                                ],
                            )
                            at = at_pool.tile([128, 512], F32R, tag="at")
                            nc.vector.tensor_mul(at[:], ch[:], attc[jb][:, ih * 512 : (ih + 1) * 512])
                            first = k == 0 and jb == 0
                            last = k == K - 1 and jb == NJB - 1
                            for tob in range(TH):
                                nc.tensor.matmul(
                                    ops[tob],
                                    xw_sb[:, k, jb, tob * 128 : (tob + 1) * 128],
                                    at[:],
                                    start=first,
                                    stop=last,
                                )
                    for tob in range(TH):
                        ot = out_pool.tile([128, 512], F32, tag="ot")
                        if tob % 2 == 0:
                            nc.vector.tensor_relu(ot[:], ops[tob])
                        else:
                            nc.scalar.activation(
                                ot[:], ops[tob], mybir.ActivationFunctionType.Relu
                            )
                        nc.sync.dma_start(out_d[lb, tob, ih], ot[:])

    nc.compile()
    return nc


_NC = None


def _get_nc():
    global _NC
    if _NC is None:
        _NC = _build_nc()
    return _NC


def _prep_in_maps(x, spatial_attention, cheb, Theta):
    x = np.ascontiguousarray(x, np.float32)
    att = np.ascontiguousarray(spatial_attention, np.float32)
    cheb = np.ascontiguousarray(cheb, np.float32)
    Theta = np.ascontiguousarray(Theta, np.float32)

    # xt[b, tl, f + 64*th, j] = x[b, th*6+tl, j, f]
    xt = (
        x.transpose(0, 1, 3, 2)                  # [B,T,F,N]
        .reshape(B, 2, TH, F, N)                 # [B,th,tl,F,N]
        .transpose(0, 2, 1, 3, 4)                # [B,tl,th,F,N]
        .reshape(B, TH, 128, N)
    )
    xt = _round_tf32(xt)

    attT = np.ascontiguousarray(att.transpose(0, 2, 1))
    chebT = np.ascontiguousarray(cheb.transpose(0, 2, 1))

    theta = np.zeros((128, THETA_PAD), np.float32)
    cat = np.concatenate([Theta[k] for k in range(K)], axis=1)  # [64, 192]
    theta[0:64, 0 : K * F] = cat
    theta[64:128, 256 : 256 + K * F] = cat
    theta = _round_tf32(theta)

    in_maps = []
    for c in range(NCORES):
        in_maps.append(
            {
                "xt": np.ascontiguousarray(xt[c * BPC : (c + 1) * BPC]),
                "attT": np.ascontiguousarray(attT[c * BPC : (c + 1) * BPC]),
                "chebT": chebT,
                "theta": theta,
            }
        )
    return in_maps


def _assemble_out(results):
    out = np.empty((B, T, N, F), np.float32)
    for c in range(NCORES):
        dev = results[c]["out"]  # [BPC, TH, 2, 128, 512]
        for lb in range(BPC):
            b = c * BPC + lb
            # dev[tob, ih, th*64+o, c] -> out[th*6+tob, ih*512+c, o]
            out[b] = (
                dev[lb]
                .reshape(TH, 2, 2, 64, 512)      # [tob, ih, th, o, ic]
                .transpose(2, 0, 1, 4, 3)        # [th, tob, ih, ic, o]
                .reshape(T, N, F)
            )
    return out


def kernel(x, spatial_attention, cheb, Theta):
    nc = _get_nc()
    in_maps = _prep_in_maps(x, spatial_attention, cheb, Theta)
    try:
        res = run_bass_kernel_spmd(nc, in_maps, list(range(NCORES)))
    except Exception:
        # transient device/runtime hiccups have been observed once per boot;
        # a single retry on a fresh execute has always succeeded
        res = run_bass_kernel_spmd(nc, in_maps, list(range(NCORES)))
    return _assemble_out(res.results)


# revision 18
# speedup vs baseline: 1.0326x; 1.0326x over previous
"""ChebConv with spatial attention — Trainium2 Bass kernel.

Problem (reference semantics):
    A   = cheb[None,k] * spatial_attention[b]          # [B,K,N,N]
    rhs = einsum('bkij,btjf->btkif', A, x)             # graph propagation
    out = relu(einsum('btkif,kfo->btio', rhs, Theta))  # per-order linear + sum_k

Shapes: B=16, T=12, N=1024, F_in=F_out=64, K=3 (all fp32).

Strategy:
  * Data-parallel over B: 2 batches per core x 8 NeuronCores. No collectives.
  * Theta applied FIRST:  XW_bk[j,(t,o)] = x[b,t,j,:] @ Theta[k]   (cheap)
    then the 77-GFLOP graph propagation, computed TRANSPOSED so that the
    reusable XW tiles are the stationary operand:
        out'[(t,o), i] = sum_{k,j} XW[j,(t,o)] * AT[j,i]
    with AT[j,i] = chebT[j,i]*attT[j,i] (elementwise, vector engine).
    att/cheb/x are transposed on the host (free layout prep).
  * float32r (tf32) matmuls: 4x faster than fp32 on the PE at free-dim>=256,
    ~3e-4 relative error.  Operands are pre-rounded (host) or rounded on
    write by DVE/ACT, as the hardware requires.
  * XW-phase matmuls are packed in concurrent PE row groups (t, t+6).
  * chebT is streamed from HBM in [128,512] chunks; x, attT, XW resident.
  * out' is stored transposed+permuted; the host un-permutes (numpy).
"""
import numpy as np

import concourse.bass as bass
import concourse.tile as tile
from concourse import bacc, mybir
from concourse.bass_utils import run_bass_kernel_spmd

F32 = mybir.dt.float32
F32R = mybir.dt.float32r

B, T, N, F, K = 16, 12, 1024, 64, 3
NCORES = 8
BPC = B // NCORES          # batches per core
TH = T // 2                # 6 "tl" column chunks of x
TF = T * F                 # 768 = 6 to'-blocks of 128
NJB = N // 128             # 8 j-blocks
THETA_PAD = 512            # block-diag: rows 0:64 -> cols 0:256, rows 64:128 -> cols 256:512


def _round_tf32(a: np.ndarray) -> np.ndarray:
    """Round fp32 -> tf32 (10 mantissa bits), required for float32r operands."""
    u = np.ascontiguousarray(a).view(np.uint32)
    lsb = (u >> np.uint32(13)) & np.uint32(1)
    rounded = (u + np.uint32(0x0FFF) + lsb) & np.uint32(0xFFFFE000)
    return rounded.view(np.float32)


def _build_nc():
    nc = bacc.Bacc("TRN2", target_bir_lowering=False, debug=False, num_devices=NCORES)

    # xt[b, tl, f + 64*th, j] = x[b, th*6+tl, j, f]
    xt_d = nc.dram_tensor("xt", [BPC, TH, 128, N], F32R, kind="ExternalInput")
    att_d = nc.dram_tensor("attT", [BPC, N, N], F32, kind="ExternalInput")
    cheb_d = nc.dram_tensor("chebT", [K, N, N], F32, kind="ExternalInput")
    th_d = nc.dram_tensor("theta", [128, THETA_PAD], F32R, kind="ExternalInput")
    # out'[b, tob, ih, to'-in-block, i-in-half]; to' = tl*128 + th*64 + o
    out_d = nc.dram_tensor("out", [BPC, TH, 2, 128, 512], F32, kind="ExternalOutput")

    with tile.TileContext(nc) as tc:
        with (
            tc.tile_pool(name="const", bufs=1) as const_pool,
            tc.tile_pool(name="xtc", bufs=9) as xtc_pool,
            tc.tile_pool(name="attc", bufs=10) as attc_pool,
            tc.tile_pool(name="xw", bufs=1) as xw_pool,
            tc.tile_pool(name="cheb", bufs=8) as cheb_pool,
            tc.tile_pool(name="atp", bufs=8) as at_pool,
            tc.tile_pool(name="outp", bufs=6) as out_pool,
            tc.tile_pool(name="ps", bufs=4, space="PSUM") as ps_pool,
        ):
            theta_sb = const_pool.tile([128, THETA_PAD], F32R)
            nc.sync.dma_start(theta_sb[:], th_d[:])

            # PE warmup: ~3.5us of dummy matmuls so HAM un-throttles before
            # the real work starts (runs during the xt/att input DMAs).
            warm = ps_pool.tile([128, 1024], F32, tag="ps", name="warm")
            for w in range(10):
                nc.tensor.matmul(
                    warm[:, 0:128],
                    theta_sb[:, 0:128],
                    theta_sb[:, 0:128],
                    start=True,
                    stop=True,
                )

            for lb in range(BPC):
                # chunked input loads (x first: it gates the XW phase)
                xtc = []
                for tl in range(TH):
                    xc = xtc_pool.tile([128, N], F32R, tag="xt", name=f"xt{lb}_{tl}")
                    nc.sync.dma_start(xc[:], xt_d[lb, tl])
                    xtc.append(xc)
                attc = []
                for jb in range(NJB):
                    ac = attc_pool.tile([128, N], F32, tag="att", name=f"att{lb}_{jb}")
                    nc.sync.dma_start(
                        ac[:], att_d[lb, jb * 128 : (jb + 1) * 128, :]
                    )
                    attc.append(ac)

                # ---- XW phase: xw[j, k, jb, to'] with to' = tl*128+th*64+o
                xw_sb = xw_pool.tile([128, K, NJB, TF], F32R, tag="xw", name=f"xw{lb}")
                for tl2 in range(TH // 2):
                    for jb in range(NJB):
                        pxw = ps_pool.tile(
                            [128, 1024], F32, tag="ps", name=f"pxw{lb}_{jb}_{tl2}"
                        )
                        for h in range(2):
                            nc.tensor.matmul(
                                pxw[:, h * 512 : (h + 1) * 512],
                                xtc[tl2 * 2 + h][:, jb * 128 : (jb + 1) * 128],
                                theta_sb[:],
                                start=True,
                                stop=True,
                            )
                        src = pxw[:].rearrange(
                            "p (a t k o) -> p k a t o", a=2, t=2, k=4, o=64
                        )[:, 0:K]
                        dst = xw_sb[:, :, jb, tl2 * 256 : (tl2 + 1) * 256].rearrange(
                            "p k (a t o) -> p k a t o", a=2, t=2
                        )
                        if tl2 == TH // 2 - 1:
                            # final XW pass: half-copies on both engines so the
                            # psum slots drain fast for the main phase
                            nc.scalar.copy(dst[:, :, 0], src[:, :, 0])
                            nc.vector.tensor_copy(dst[:, :, 1], src[:, :, 1])
                        elif jb % 2 == 0:
                            nc.scalar.copy(dst, src)
                        else:
                            nc.vector.tensor_copy(dst, src)
                        last_pxw = pxw

                # ---- main phase: out'[to', i] = sum_{k,j} XW[j,to']*AT[j,i]
                for ih in range(2):
                    opt2 = [
                        ps_pool.tile([128, 1024], F32, tag="ps", name=f"op_{lb}_{ih}_{i}")
                        for i in range(TH // 2)
                    ]
                    ops = [opt2[i // 2][:, (i % 2) * 512 : (i % 2 + 1) * 512] for i in range(TH)]
                    for k in range(K):
                        for jb in range(NJB):
                            ch = cheb_pool.tile([128, 512], F32, tag="ch")
                            nc.sync.dma_start(
                                ch[:],
                                cheb_d[
                                    k,
                                    jb * 128 : (jb + 1) * 128,
                                    ih * 512 : (ih + 1) * 512,
                                ],
                            )
                            at = at_pool.tile([128, 512], F32R, tag="at")
                            nc.vector.tensor_mul(at[:], ch[:], attc[jb][:, ih * 512 : (ih + 1) * 512])
                            first = k == 0 and jb == 0
                            last = k == K - 1 and jb == NJB - 1
                            for tob in range(TH):
                                nc.tensor.matmul(
                                    ops[tob],
                                    xw_sb[:, k, jb, tob * 128 : (tob + 1) * 128],
                                    at[:],
                                    start=first,
                                    stop=last,
                                )
                    for tob in range(TH):
                        ot = out_pool.tile([128, 512], F32, tag="ot")
                        if tob % 2 == 0:
                            nc.vector.tensor_relu(ot[:], ops[tob])
                        else:
                            nc.scalar.activation(
                                ot[:], ops[tob], mybir.ActivationFunctionType.Relu
                            )
                        nc.sync.dma_start(out_d[lb, tob, ih], ot[:])

    nc.compile()
    return nc


_NC = None


def _get_nc():
    global _NC
    if _NC is None:
        _NC = _build_nc()
    return _NC


def _prep_in_maps(x, spatial_attention, cheb, Theta):
    x = np.ascontiguousarray(x, np.float32)
    att = np.ascontiguousarray(spatial_attention, np.float32)
    cheb = np.ascontiguousarray(cheb, np.float32)
    Theta = np.ascontiguousarray(Theta, np.float32)

    # xt[b, tl, f + 64*th, j] = x[b, th*6+tl, j, f]
    xt = (
        x.transpose(0, 1, 3, 2)                  # [B,T,F,N]
        .reshape(B, 2, TH, F, N)                 # [B,th,tl,F,N]
        .transpose(0, 2, 1, 3, 4)                # [B,tl,th,F,N]
        .reshape(B, TH, 128, N)
    )
    xt = _round_tf32(xt)

    attT = np.ascontiguousarray(att.transpose(0, 2, 1))
    chebT = np.ascontiguousarray(cheb.transpose(0, 2, 1))

    theta = np.zeros((128, THETA_PAD), np.float32)
    cat = np.concatenate([Theta[k] for k in range(K)], axis=1)  # [64, 192]
    theta[0:64, 0 : K * F] = cat
    theta[64:128, 256 : 256 + K * F] = cat
    theta = _round_tf32(theta)

    in_maps = []
    for c in range(NCORES):
        in_maps.append(
            {
                "xt": np.ascontiguousarray(xt[c * BPC : (c + 1) * BPC]),
                "attT": np.ascontiguousarray(attT[c * BPC : (c + 1) * BPC]),
                "chebT": chebT,
                "theta": theta,
            }
        )
    return in_maps


def _assemble_out(results):
    out = np.empty((B, T, N, F), np.float32)
    for c in range(NCORES):
        dev = results[c]["out"]  # [BPC, TH, 2, 128, 512]
        for lb in range(BPC):
            b = c * BPC + lb
            # dev[tob, ih, th*64+o, c] -> out[th*6+tob, ih*512+c, o]
            out[b] = (
                dev[lb]
                .reshape(TH, 2, 2, 64, 512)      # [tob, ih, th, o, ic]
                .transpose(2, 0, 1, 4, 3)        # [th, tob, ih, ic, o]
                .reshape(T, N, F)
            )
    return out


def kernel(x, spatial_attention, cheb, Theta):
    nc = _get_nc()
    in_maps = _prep_in_maps(x, spatial_attention, cheb, Theta)
    try:
        res = run_bass_kernel_spmd(nc, in_maps, list(range(NCORES)))
    except Exception:
        # transient device/runtime hiccups have been observed once per boot;
        # a single retry on a fresh execute has always succeeded
        res = run_bass_kernel_spmd(nc, in_maps, list(range(NCORES)))
    return _assemble_out(res.results)
